# revision 3
# baseline (speedup 1.0000x reference)
"""Causal attention-matrix kernel for Trainium2 (Bass/Tile), 8-core SPMD.

Problem: out[b] = softmax((Q[b] @ K[b].T + causal_mask) / sqrt(S_k), axis=-1)
with B=8, S=2048, D=512, fp32 in/out.

Strategy:
- Data-parallel over batch: core b handles batch b (no communication).
- Host pre-transposes Q,K to [D, S] and casts to bf16 so the device matmul
  (out = lhsT.T @ rhs, contraction along the partition dim) needs no on-device
  transposes.  PSUM accumulates in fp32.
- Causality: for q-block i (128 rows) only k <= 128*(i+1) is computed/written;
  the strictly-upper blocks are never touched (output buffers are zero-donated,
  so they stay exactly 0).  The diagonal 128x128 block gets an additive -1e10
  mask before exp (kills the sum contribution) and a multiplicative 0/1 mask
  after exp (exact zeros).
- Softmax skips the max-subtraction: logits ~ N(0, 0.5); |logit| < ~4, so fp32
  exp cannot overflow.  exp runs on ScalarE with accum_out producing the row
  sums for free; VectorE applies the reciprocal scale.
"""

import math
from contextlib import ExitStack

import ml_dtypes
import numpy as np

import concourse.bass as bass
import concourse.tile as tile
from concourse import mybir
from concourse.bass_utils import run_bass_kernel_spmd
from concourse.masks import make_causal_mask

B, S, D = 8, 2048, 512
P = 128
ND = D // P  # 4 contraction tiles
NB = S // P  # 16 q-blocks
BANK = 512  # PSUM bank width in fp32
SCALE = 1.0 / math.sqrt(float(S))
NEG = -1e10

_NC_CACHE = None


def _emit(ctx: ExitStack, tc: "tile.TileContext", out, qt, kt):
    nc = tc.nc

    consts = ctx.enter_context(tc.tile_pool(name="consts", bufs=1))
    psum = ctx.enter_context(tc.tile_pool(name="psum", bufs=2, space="PSUM"))
    exps = ctx.enter_context(tc.tile_pool(name="exps", bufs=3))
    stats = ctx.enter_context(tc.tile_pool(name="stats", bufs=8))

    # Whole Q^T / K^T resident in SBUF: [128, 4, 2048] bf16 = 16KB/partition each.
    qts = consts.tile([P, ND, S], mybir.dt.bfloat16)
    kts = consts.tile([P, ND, S], mybir.dt.bfloat16)
    for d in range(ND):
        nc.sync.dma_start(out=qts[:, d, :], in_=qt[P * d : P * (d + 1), :])
        nc.sync.dma_start(out=kts[:, d, :], in_=kt[P * d : P * (d + 1), :])

    # Additive causal mask for the diagonal block: 0 on/below diag, NEG above.
    addmask = consts.tile([P, P], mybir.dt.float32)
    make_causal_mask(nc, addmask, mask_val=NEG)
    # Multiplicative 0/1 mask: 1 on/below diag, 0 above (exact zeros post-exp).
    mulmask = consts.tile([P, P], mybir.dt.float32)
    nc.gpsimd.memset(mulmask, 1.0)
    nc.gpsimd.affine_select(
        out=mulmask,
        in_=mulmask,
        compare_op=mybir.AluOpType.is_ge,
        fill=0.0,
        base=0,
        pattern=[[-1, P]],
        channel_multiplier=1,
    )

    for i in range(NB):
        wi = P * (i + 1)  # valid (causal) width for this q-block
        nbanks = (wi + BANK - 1) // BANK
        ps = psum.tile([P, 4 * BANK], mybir.dt.float32, tag="ps")
        # Q.K^T for this q-block, accumulated over the 4 contraction tiles.
        # Bank-major so each PSUM bank sees a clean start..stop group.
        for c in range(nbanks):
            for d in range(ND):
                nc.tensor.matmul(
                    ps[:, BANK * c : BANK * (c + 1)],
                    qts[:, d, P * i : P * (i + 1)],  # stationary [128d, 128q]
                    kts[:, d, BANK * c : BANK * (c + 1)],  # moving [128d, 512k]
                    start=(d == 0),
                    stop=(d == ND - 1),
                )
        # Mask the diagonal block (additive, pre-exp: keeps row sums correct).
        nc.vector.tensor_add(ps[:, wi - P : wi], ps[:, wi - P : wi], addmask)
        # exp(scale * s) from PSUM -> SBUF with row-sum accumulation on ScalarE.
        ex = exps.tile([P, S], mybir.dt.float32, tag="ex")
        sm = stats.tile([P, 1], mybir.dt.float32, tag="sm")
        nc.scalar.activation(
            out=ex[:, :wi],
            in_=ps[:, :wi],
            func=mybir.ActivationFunctionType.Exp,
            bias=0.0,
            scale=float(SCALE),
            accum_out=sm,
        )
        # Exact zeros above the diagonal.
        nc.vector.tensor_mul(ex[:, wi - P : wi], ex[:, wi - P : wi], mulmask)
        rc = stats.tile([P, 1], mybir.dt.float32, tag="rc")
        nc.vector.reciprocal(rc, sm)
        nc.vector.tensor_scalar_mul(ex[:, :wi], ex[:, :wi], rc)
        nc.sync.dma_start(out=out[P * i : P * (i + 1), 0:wi], in_=ex[:, :wi])


def _split_multi_waits(nc: "bass.Bass") -> None:
    """The walrus build here encodes at most ONE sync-wait command per
    instruction; Tile freely emits several.  Hoist all but the last wait of
    each instruction onto single-wait EventSemaphore instructions inserted
    just before it on the same engine (sequencers execute in program order,
    so sequential single waits are equivalent to one multi-wait)."""
    for f in nc.m.functions:
        for bb in f.blocks:
            new: list = []
            changed = False
            for inst in bb.instructions:
                si = inst.sync_info
                waits = list(si.on_wait) if si is not None and si.on_wait else []
                if len(waits) > 1:
                    changed = True
                    for w in waits[:-1]:
                        ev = mybir.InstEventSemaphore(
                            name=nc.get_next_instruction_name(), ins=[], outs=[]
                        )
                        ev.engine = inst.engine
                        ev.sync_info = mybir.SyncInfo(on_wait=[w], on_update=[])
                        new.append(ev)
                    inst.sync_info = mybir.SyncInfo(
                        on_wait=[waits[-1]],
                        on_update=list(si.on_update) if si.on_update else [],
                    )
                new.append(inst)
            if changed:
                bb.instructions = new


def build_bass(split_waits: bool = True) -> "bass.Bass":
    nc = bass.Bass(trn_type="TRN2", target_bir_lowering=False, debug=False)
    qt = nc.dram_tensor("qt", [D, S], mybir.dt.bfloat16, kind="ExternalInput").ap()
    kt = nc.dram_tensor("kt", [D, S], mybir.dt.bfloat16, kind="ExternalInput").ap()
    out = nc.dram_tensor("out", [S, S], mybir.dt.float32, kind="ExternalOutput").ap()
    with tile.TileContext(nc) as tc:
        with ExitStack() as ctx:
            _emit(ctx, tc, out, qt, kt)
    if split_waits:
        # CoreSim's race detector can't model hand-inserted EventSemaphores;
        # build with split_waits=False for simulation.
        _split_multi_waits(nc)
    return nc


def kernel(K: np.ndarray, Q: np.ndarray) -> np.ndarray:
    K = np.asarray(K)
    Q = np.asarray(Q)
    assert Q.shape == (B, S, D) and K.shape == (B, S, D), (Q.shape, K.shape)

    bf16 = ml_dtypes.bfloat16
    # Host prep: cast to bf16 and lay out as [B, D, S] so the device needs no
    # transposes (matmul contracts along the partition dim of both operands).
    qt_all = np.ascontiguousarray(Q.astype(bf16).transpose(0, 2, 1))
    kt_all = np.ascontiguousarray(K.astype(bf16).transpose(0, 2, 1))

    global _NC_CACHE
    if _NC_CACHE is None:
        _NC_CACHE = build_bass()
    nc = _NC_CACHE

    in_maps = [{"qt": qt_all[b], "kt": kt_all[b]} for b in range(B)]
    res = run_bass_kernel_spmd(nc, in_maps, core_ids=list(range(B)))
    out = np.stack([res.results[b]["out"] for b in range(B)], axis=0)
    return out


if __name__ == "__main__":
    nc = build_bass()
    print("built OK; instructions:", sum(1 for _ in nc.m.functions[0].basicblocks[0].instructions) if hasattr(nc.m.functions[0], "basicblocks") else "?")


# revision 17
# speedup vs baseline: 1.4043x; 1.4043x over previous
"""Causal attention-matrix kernel for Trainium2 (Bass/Tile), 8-core SPMD.

Problem: out[b] = softmax((Q[b] @ K[b].T + causal_mask) / sqrt(S_k), axis=-1)
with B=8, S=2048, D=512, fp32 in/out.

Strategy:
- Data-parallel over batch: core b handles batch b (no communication).
- Host pre-transposes Q,K to [D, S] and casts to bf16 so the device matmul
  (out = lhsT.T @ rhs, contraction along the partition dim) needs no on-device
  transposes.  PSUM accumulates in fp32.
- Causality: for q-block i (128 rows) only k <= 128*(i+1) is computed/written;
  the strictly-upper blocks are never touched (output buffers are zero-donated,
  so they stay exactly 0).  The diagonal 128x128 block gets an additive -1e10
  mask before exp (kills the sum contribution) and a multiplicative 0/1 mask
  after exp (exact zeros).
- Softmax skips the max-subtraction: logits ~ N(0, 0.5); |logit| < ~4, so fp32
  exp cannot overflow.  exp runs on ScalarE with accum_out producing the row
  sums for free; VectorE applies the reciprocal scale.
"""

import math
from contextlib import ExitStack

import ml_dtypes
import numpy as np

import concourse.bass as bass
import concourse.tile as tile
from concourse import mybir
from concourse.bass_utils import run_bass_kernel_spmd
from concourse.masks import make_causal_mask

B, S, D = 8, 2048, 512
P = 128
ND = D // P  # 4 contraction tiles
NB = S // P  # 16 q-blocks
BANK = 512  # PSUM bank width in fp32
SCALE = 1.0 / math.sqrt(float(S))
NEG = -1e10

_NC_CACHE = None


def _emit(ctx: ExitStack, tc: "tile.TileContext", out, qt, kt):
    nc = tc.nc

    consts = ctx.enter_context(tc.tile_pool(name="consts", bufs=1))
    # One PSUM bank per (block, k-chunk): exp consumes chunks right behind the
    # PE, so up to 8 chunks are in flight and PE never waits on a whole
    # block's softmax.
    psum = ctx.enter_context(tc.tile_pool(name="psum", bufs=8, space="PSUM"))
    # Enough exp buffers that ACT never waits on an output store to free a
    # slot (stores can lag several blocks behind).
    exps = ctx.enter_context(tc.tile_pool(name="exps", bufs=8))
    stats = ctx.enter_context(tc.tile_pool(name="stats", bufs=16))

    # Whole Q^T / K^T resident in SBUF: [128, 4, 2048] bf16 = 16KB/partition each.
    qts = consts.tile([P, ND, S], mybir.dt.bfloat16)
    kts = consts.tile([P, ND, S], mybir.dt.bfloat16)
    # Load in 3 column waves (bank 0 -> blocks 0-3 start early; bank 1 ->
    # blocks 4-7; the rest -> the big blocks).  One 3D-AP DMA per tensor per
    # wave keeps the DMA instruction count low (each costs ~0.6us of HWDGE
    # queue time).
    qt3 = qt.rearrange("(n p) s -> p n s", p=P)
    kt3 = kt.rearrange("(n p) s -> p n s", p=P)
    # First wave split by contraction-half so block 0's d0/d1 matmuls start
    # ~1.5us sooner.
    for d0, d1 in ((0, 2), (2, ND)):
        nc.sync.dma_start(out=qts[:, d0:d1, 0:BANK], in_=qt3[:, d0:d1, 0:BANK])
        nc.sync.dma_start(out=kts[:, d0:d1, 0:BANK], in_=kt3[:, d0:d1, 0:BANK])
    for c0, c1 in ((BANK, 2 * BANK), (2 * BANK, S)):
        nc.sync.dma_start(out=kts[:, :, c0:c1], in_=kt3[:, :, c0:c1])
        nc.sync.dma_start(out=qts[:, :, c0:c1], in_=qt3[:, :, c0:c1])

    # Additive causal mask for the diagonal block: 0 on/below diag, NEG above.
    # exp(scale*(s+NEG)) underflows to exact +0.0 on the ACT spline (verified
    # on HW: exp(x)=0x0 for x <= -104), matching the reference's exact zeros.
    addmask = consts.tile([P, P], mybir.dt.float32)
    make_causal_mask(nc, addmask, mask_val=NEG)

    # Ascending through the bank-0/1 blocks (data-ready earliest, PE warms up
    # while the rest of K^T/Q^T loads), then descending through the big
    # blocks; finish on tiny block 3 so the post-PE tail (exp+scale+store of
    # the last block) is as short as possible.
    order = [0, 1, 2, 4, 5, 6, 7] + list(range(NB - 1, 7, -1)) + [3]
    for i in order:
        wi = P * (i + 1)  # valid (causal) width for this q-block
        nbanks = (wi + BANK - 1) // BANK
        ex = exps.tile([P, S], mybir.dt.float32, tag="ex")
        sums = stats.tile([P, ND], mybir.dt.float32, tag="sums")
        # Q.K^T chunk by PSUM bank; each chunk is exp'd (with per-chunk row
        # sums) as soon as its 4-deep accumulation finishes.  The last chunk
        # is truncated to the causal width and additively masked on its
        # diagonal 128 columns before exp (exp underflows to exact 0).
        for c in range(nbanks):
            c0 = BANK * c
            cw = min(BANK, wi - c0)
            ps = psum.tile([P, BANK], mybir.dt.float32, tag="ps")
            for d in range(ND):
                nc.tensor.matmul(
                    ps[:, :cw],
                    qts[:, d, P * i : P * (i + 1)],  # stationary [128d, 128q]
                    kts[:, d, c0 : c0 + cw],  # moving [128d, <=512k]
                    start=(d == 0),
                    stop=(d == ND - 1),
                )
            if c == nbanks - 1:
                nc.vector.tensor_add(ps[:, cw - P : cw], ps[:, cw - P : cw], addmask)
            nc.scalar.activation(
                out=ex[:, c0 : c0 + cw],
                in_=ps[:, :cw],
                func=mybir.ActivationFunctionType.Exp,
                bias=0.0,
                scale=float(SCALE),
                accum_out=sums[:, c : c + 1],
            )
        rc = stats.tile([P, 1], mybir.dt.float32, tag="rc")
        if nbanks == 1:
            nc.vector.reciprocal(rc, sums[:, 0:1])
        else:
            tot = stats.tile([P, 1], mybir.dt.float32, tag="tot")
            nc.vector.reduce_sum(tot, sums[:, :nbanks], axis=mybir.AxisListType.X)
            nc.vector.reciprocal(rc, tot)
        nc.vector.tensor_scalar_mul(ex[:, :wi], ex[:, :wi], rc)
        # One big store per block (each DMA instruction costs ~0.6us of HWDGE
        # queue time).  SP dispatch: ACT's sequencer is busy with the exps,
        # and with 8 exp buffers a store may lag the compute harmlessly.
        nc.sync.dma_start(out=out[P * i : P * (i + 1), 0:wi], in_=ex[:, :wi])


def _split_multi_waits(nc: "bass.Bass") -> None:
    """The walrus build here encodes at most ONE sync-wait command per
    instruction; Tile freely emits several.  Hoist all but the last wait of
    each instruction onto single-wait EventSemaphore instructions inserted
    just before it on the same engine (sequencers execute in program order,
    so sequential single waits are equivalent to one multi-wait)."""
    for f in nc.m.functions:
        for bb in f.blocks:
            new: list = []
            changed = False
            for inst in bb.instructions:
                si = inst.sync_info
                waits = list(si.on_wait) if si is not None and si.on_wait else []
                if len(waits) > 1:
                    changed = True
                    for w in waits[:-1]:
                        ev = mybir.InstEventSemaphore(
                            name=nc.get_next_instruction_name(), ins=[], outs=[]
                        )
                        ev.engine = inst.engine
                        ev.sync_info = mybir.SyncInfo(on_wait=[w], on_update=[])
                        new.append(ev)
                    inst.sync_info = mybir.SyncInfo(
                        on_wait=[waits[-1]],
                        on_update=list(si.on_update) if si.on_update else [],
                    )
                new.append(inst)
            if changed:
                bb.instructions = new


def build_bass(split_waits: bool = True) -> "bass.Bass":
    nc = bass.Bass(trn_type="TRN2", target_bir_lowering=False, debug=False)
    qt = nc.dram_tensor("qt", [D, S], mybir.dt.bfloat16, kind="ExternalInput").ap()
    kt = nc.dram_tensor("kt", [D, S], mybir.dt.bfloat16, kind="ExternalInput").ap()
    out = nc.dram_tensor("out", [S, S], mybir.dt.float32, kind="ExternalOutput").ap()
    with tile.TileContext(nc) as tc:
        with ExitStack() as ctx:
            _emit(ctx, tc, out, qt, kt)
    if split_waits:
        # CoreSim's race detector can't model hand-inserted EventSemaphores;
        # build with split_waits=False for simulation.
        _split_multi_waits(nc)
    return nc


def kernel(K: np.ndarray, Q: np.ndarray) -> np.ndarray:
    K = np.asarray(K)
    Q = np.asarray(Q)
    assert Q.shape == (B, S, D) and K.shape == (B, S, D), (Q.shape, K.shape)

    bf16 = ml_dtypes.bfloat16
    # Host prep: cast to bf16 and lay out as [B, D, S] so the device needs no
    # transposes (matmul contracts along the partition dim of both operands).
    qt_all = np.ascontiguousarray(Q.astype(bf16).transpose(0, 2, 1))
    kt_all = np.ascontiguousarray(K.astype(bf16).transpose(0, 2, 1))

    global _NC_CACHE
    if _NC_CACHE is None:
        _NC_CACHE = build_bass()
    nc = _NC_CACHE

    in_maps = [{"qt": qt_all[b], "kt": kt_all[b]} for b in range(B)]
    res = run_bass_kernel_spmd(nc, in_maps, core_ids=list(range(B)))
    out = np.stack([res.results[b]["out"] for b in range(B)], axis=0)
    return out


if __name__ == "__main__":
    nc = build_bass()
    print("built OK; instructions:", sum(1 for _ in nc.m.functions[0].basicblocks[0].instructions) if hasattr(nc.m.functions[0], "basicblocks") else "?")


# revision 22
# speedup vs baseline: 1.4122x; 1.0056x over previous
"""Causal attention-matrix kernel for Trainium2 (Bass/Tile), 8-core SPMD.

Problem: out[b] = softmax((Q[b] @ K[b].T + causal_mask) / sqrt(S_k), axis=-1)
with B=8, S=2048, D=512, fp32 in/out.

Strategy:
- Data-parallel over batch: core b handles batch b (no communication).
- Host pre-transposes Q,K to [D, S] and casts to bf16 so the device matmul
  (out = lhsT.T @ rhs, contraction along the partition dim) needs no on-device
  transposes.  PSUM accumulates in fp32.
- Causality: for q-block i (128 rows) only k <= 128*(i+1) is computed/written;
  the strictly-upper blocks are never touched (output buffers are zero-donated,
  so they stay exactly 0).  The diagonal 128x128 block gets an additive -1e10
  mask before exp; ACT's exp underflows to exact +0.0 there (HW-verified), so
  both the row sums and the stored zeros match the reference exactly.
- Softmax skips the max-subtraction: logits ~ N(0, 0.5); |logit| < ~4, so fp32
  exp cannot overflow.  exp runs on ScalarE chunk-by-PSUM-bank with accum_out
  producing the row sums for free; VectorE applies the reciprocal scale.
- Modeled per-core exec (cost-model timeline): ~43.6us; DMA busy 36.4us
  (12.9MB @ ~350GB/s) and PE busy ~31us -- at the memory/compute ridge.
"""

import math
from contextlib import ExitStack

import ml_dtypes
import numpy as np

import concourse.bass as bass
import concourse.tile as tile
from concourse import mybir
from concourse.bass_utils import run_bass_kernel_spmd
from concourse.masks import make_causal_mask

B, S, D = 8, 2048, 512
P = 128
ND = D // P  # 4 contraction tiles
NB = S // P  # 16 q-blocks
BANK = 512  # PSUM bank width in fp32
SCALE = 1.0 / math.sqrt(float(S))
NEG = -1e10

_NC_CACHE = None


def _emit(ctx: ExitStack, tc: "tile.TileContext", out, qt, kt):
    nc = tc.nc

    consts = ctx.enter_context(tc.tile_pool(name="consts", bufs=1))
    # One PSUM bank per (block, k-chunk): exp consumes chunks right behind the
    # PE, so up to 8 chunks are in flight and PE never waits on a whole
    # block's softmax.
    psum = ctx.enter_context(tc.tile_pool(name="psum", bufs=8, space="PSUM"))
    # Enough exp buffers that ACT never waits on an output store to free a
    # slot (stores can lag several blocks behind).
    exps = ctx.enter_context(tc.tile_pool(name="exps", bufs=8))
    stats = ctx.enter_context(tc.tile_pool(name="stats", bufs=16))

    # Whole Q^T / K^T resident in SBUF: [128, 4, 2048] bf16 = 16KB/partition each.
    qts = consts.tile([P, ND, S], mybir.dt.bfloat16)
    kts = consts.tile([P, ND, S], mybir.dt.bfloat16)
    # Load in 3 column waves (bank 0 -> blocks 0-3 start early; bank 1 ->
    # blocks 4-7; the rest -> the big blocks).  One 3D-AP DMA per tensor per
    # wave keeps the DMA instruction count low (each costs ~0.6us of HWDGE
    # queue time).
    qt3 = qt.rearrange("(n p) s -> p n s", p=P)
    kt3 = kt.rearrange("(n p) s -> p n s", p=P)
    # First wave split by contraction-half so block 0's d0/d1 matmuls start
    # ~1.5us sooner.
    for d0, d1 in ((0, 2), (2, ND)):
        nc.sync.dma_start(out=qts[:, d0:d1, 0:BANK], in_=qt3[:, d0:d1, 0:BANK])
        nc.sync.dma_start(out=kts[:, d0:d1, 0:BANK], in_=kt3[:, d0:d1, 0:BANK])
    # Then: bank-1 columns (blocks 4-7), the top Q^T columns (so block 15's
    # early k-chunks can start), the rest of K^T, the remaining Q^T columns.
    for t3, src, c0, c1 in (
        (kts, kt3, BANK, 2 * BANK),
        (qts, qt3, BANK, 2 * BANK),
        (qts, qt3, 3 * BANK, S),
        (kts, kt3, 2 * BANK, S),
        (qts, qt3, 2 * BANK, 3 * BANK),
    ):
        nc.sync.dma_start(out=t3[:, :, c0:c1], in_=src[:, :, c0:c1])

    # Additive causal mask for the diagonal block: 0 on/below diag, NEG above.
    # exp(scale*(s+NEG)) underflows to exact +0.0 on the ACT spline (verified
    # on HW: exp(x)=0x0 for x <= -104), matching the reference's exact zeros.
    addmask = consts.tile([P, P], mybir.dt.float32)
    make_causal_mask(nc, addmask, mask_val=NEG)

    # Ascending through the bank-0/1 blocks (data-ready earliest, PE warms up
    # while the rest of K^T/Q^T loads), then descending through the big
    # blocks; finish on tiny block 0 so the post-PE tail (exp+scale+store of
    # the last block) is as short as possible.
    order = [1, 2, 3, 4, 5, 6, 7] + list(range(NB - 1, 7, -1)) + [0]
    for i in order:
        wi = P * (i + 1)  # valid (causal) width for this q-block
        nbanks = (wi + BANK - 1) // BANK
        ex = exps.tile([P, S], mybir.dt.float32, tag="ex")
        sums = stats.tile([P, ND], mybir.dt.float32, tag="sums")
        # Q.K^T chunk by PSUM bank; each chunk is exp'd (with per-chunk row
        # sums) as soon as its 4-deep accumulation finishes.  The last chunk
        # is truncated to the causal width and additively masked on its
        # diagonal 128 columns before exp (exp underflows to exact 0).
        for c in range(nbanks):
            c0 = BANK * c
            cw = min(BANK, wi - c0)
            ps = psum.tile([P, BANK], mybir.dt.float32, tag="ps")
            for d in range(ND):
                nc.tensor.matmul(
                    ps[:, :cw],
                    qts[:, d, P * i : P * (i + 1)],  # stationary [128d, 128q]
                    kts[:, d, c0 : c0 + cw],  # moving [128d, <=512k]
                    start=(d == 0),
                    stop=(d == ND - 1),
                )
            if c == nbanks - 1:
                nc.vector.tensor_add(ps[:, cw - P : cw], ps[:, cw - P : cw], addmask)
            nc.scalar.activation(
                out=ex[:, c0 : c0 + cw],
                in_=ps[:, :cw],
                func=mybir.ActivationFunctionType.Exp,
                bias=0.0,
                scale=float(SCALE),
                accum_out=sums[:, c : c + 1],
            )
        rc = stats.tile([P, 1], mybir.dt.float32, tag="rc")
        if nbanks == 1:
            nc.vector.reciprocal(rc, sums[:, 0:1])
        else:
            tot = stats.tile([P, 1], mybir.dt.float32, tag="tot")
            nc.vector.reduce_sum(tot, sums[:, :nbanks], axis=mybir.AxisListType.X)
            nc.vector.reciprocal(rc, tot)
        # One big store per block (each DMA instruction costs ~0.6us of HWDGE
        # queue time).  SP dispatch: ACT's sequencer is busy with the exps,
        # and with 8 exp buffers a store may lag the compute harmlessly.
        # The first big block processed (15) is scaled+stored in two halves so
        # its store starts while its second half is still being normalized.
        if i == NB - 1:
            h = wi // 2
            nc.vector.tensor_scalar_mul(ex[:, :h], ex[:, :h], rc)
            nc.sync.dma_start(out=out[P * i : P * (i + 1), 0:h], in_=ex[:, :h])
            nc.vector.tensor_scalar_mul(ex[:, h:wi], ex[:, h:wi], rc)
            nc.sync.dma_start(out=out[P * i : P * (i + 1), h:wi], in_=ex[:, h:wi])
        else:
            nc.vector.tensor_scalar_mul(ex[:, :wi], ex[:, :wi], rc)
            nc.sync.dma_start(out=out[P * i : P * (i + 1), 0:wi], in_=ex[:, :wi])


def _split_multi_waits(nc: "bass.Bass") -> None:
    """The walrus build here encodes at most ONE sync-wait command per
    instruction; Tile freely emits several.  Hoist all but the last wait of
    each instruction onto single-wait EventSemaphore instructions inserted
    just before it on the same engine (sequencers execute in program order,
    so sequential single waits are equivalent to one multi-wait)."""
    for f in nc.m.functions:
        for bb in f.blocks:
            new: list = []
            changed = False
            for inst in bb.instructions:
                si = inst.sync_info
                waits = list(si.on_wait) if si is not None and si.on_wait else []
                if len(waits) > 1:
                    changed = True
                    for w in waits[:-1]:
                        ev = mybir.InstEventSemaphore(
                            name=nc.get_next_instruction_name(), ins=[], outs=[]
                        )
                        ev.engine = inst.engine
                        ev.sync_info = mybir.SyncInfo(on_wait=[w], on_update=[])
                        new.append(ev)
                    inst.sync_info = mybir.SyncInfo(
                        on_wait=[waits[-1]],
                        on_update=list(si.on_update) if si.on_update else [],
                    )
                new.append(inst)
            if changed:
                bb.instructions = new


def build_bass(split_waits: bool = True) -> "bass.Bass":
    nc = bass.Bass(trn_type="TRN2", target_bir_lowering=False, debug=False)
    qt = nc.dram_tensor("qt", [D, S], mybir.dt.bfloat16, kind="ExternalInput").ap()
    kt = nc.dram_tensor("kt", [D, S], mybir.dt.bfloat16, kind="ExternalInput").ap()
    out = nc.dram_tensor("out", [S, S], mybir.dt.float32, kind="ExternalOutput").ap()
    with tile.TileContext(nc) as tc:
        with ExitStack() as ctx:
            _emit(ctx, tc, out, qt, kt)
    if split_waits:
        # CoreSim's race detector can't model hand-inserted EventSemaphores;
        # build with split_waits=False for simulation.
        _split_multi_waits(nc)
    return nc


def kernel(K: np.ndarray, Q: np.ndarray) -> np.ndarray:
    K = np.asarray(K)
    Q = np.asarray(Q)
    assert Q.shape == (B, S, D) and K.shape == (B, S, D), (Q.shape, K.shape)

    bf16 = ml_dtypes.bfloat16
    # Host prep: cast to bf16 and lay out as [B, D, S] so the device needs no
    # transposes (matmul contracts along the partition dim of both operands).
    qt_all = np.ascontiguousarray(Q.astype(bf16).transpose(0, 2, 1))
    kt_all = np.ascontiguousarray(K.astype(bf16).transpose(0, 2, 1))

    global _NC_CACHE
    if _NC_CACHE is None:
        _NC_CACHE = build_bass()
    nc = _NC_CACHE

    in_maps = [{"qt": qt_all[b], "kt": kt_all[b]} for b in range(B)]
    res = run_bass_kernel_spmd(nc, in_maps, core_ids=list(range(B)))
    out = np.stack([res.results[b]["out"] for b in range(B)], axis=0)
    return out


if __name__ == "__main__":
    nc = build_bass()
    n = sum(len(bb.instructions) for f in nc.m.functions for bb in f.blocks)
    print(f"built OK; {n} instructions")
